# revision 1
# baseline (speedup 1.0000x reference)
"""Trainium2 Bass kernel for nn_CustomAttn: fused QKV + RoPE + causal SDPA + out-proj.

Sharding: tensor-parallel over heads (16 heads / 8 cores = 2 heads/core).
Each core computes QKV for its 2 heads (d-major layouts), RoPE, causal
flash-style attention (scores kept transposed [k, q] so softmax-normalization
and the PV matmul need no per-block transposes), producing attn^T feature-major
[128, tokens]. An AllGather over the partition axis assembles the full
attn^T [1024, tokens]; each core then computes its 128-row slice of
y^T = w_out @ attn^T.  Host assembles y from the 8 row-slices.

All matmuls run in float32r (TF32-like: ~1.5e-4 rel err, 4x faster than fp32).
"""
import sys

if "/opt/trn_rl_repo" not in sys.path:
    sys.path.insert(0, "/opt/trn_rl_repo")

import numpy as np

import concourse.bass as bass
import concourse.tile as tile
from concourse import bacc, mybir
from concourse.bass_utils import run_bass_kernel_spmd
from concourse.masks import make_identity

F32 = mybir.dt.float32
F32R = mybir.dt.float32r
EXP = mybir.ActivationFunctionType.Exp

B, S, D, H, HD = 2, 2048, 1024, 16, 64
NCORE = 8
HPC = H // NCORE  # 2 heads per core
TOK = B * S  # 4096 flattened tokens
ST = 512  # s-tile / q-tile width
NST = TOK // ST  # 8
NQT = S // ST  # 4 q-tiles per batch
KB = 128  # k-block
NKB_B = S // KB  # 16 k-blocks per batch
DCH = D // 128  # 8 contraction chunks
SCALE = 1.0 / np.sqrt(HD)
ROPE_BASE = 10000.0

_CACHE: dict = {}


def _build_program(collective: bool = True):
    nc = bacc.Bacc("TRN2", target_bir_lowering=False, debug=False, num_devices=NCORE)

    # ---- DRAM I/O ----
    xT_d = nc.dram_tensor("xT", [D, TOK], F32R, kind="ExternalInput").ap()
    wq_d = nc.dram_tensor("wq", [D, 128], F32R, kind="ExternalInput").ap()
    wk_d = nc.dram_tensor("wk", [D, 128], F32R, kind="ExternalInput").ap()
    wv_d = nc.dram_tensor("wv", [D, 128], F32R, kind="ExternalInput").ap()
    wo_d = nc.dram_tensor("wo", [D, 128], F32R, kind="ExternalInput").ap()
    cos_d = nc.dram_tensor("cosT", [128, S], F32, kind="ExternalInput").ap()
    sin_d = nc.dram_tensor("sinT", [128, S], F32, kind="ExternalInput").ap()
    yt_d = nc.dram_tensor("yt", [128, TOK], F32, kind="ExternalOutput").ap()

    with tile.TileContext(nc) as tc:
        with (
            tc.tile_pool(name="const", bufs=1) as cpool,
            tc.tile_pool(name="persist", bufs=1) as ppool,
            tc.tile_pool(name="xt", bufs=2) as xpool,
            tc.tile_pool(name="rope", bufs=2) as rpool,
            tc.tile_pool(name="e", bufs=6) as epool,
            tc.tile_pool(name="at", bufs=2) as apool,
            tc.tile_pool(name="rz", bufs=2) as zpool,
            tc.tile_pool(name="agin", bufs=2) as gpool,
            tc.tile_pool(name="yt", bufs=2) as ypool,
            tc.tile_pool(name="pqkv", bufs=2, space="PSUM") as pqkv,
            tc.tile_pool(name="pscr", bufs=4, space="PSUM") as pscr,
            tc.tile_pool(name="po", bufs=2, space="PSUM") as po,
            tc.tile_pool(name="dram", bufs=1, space="DRAM") as dpool,
        ):
            # ---- constants / weights ----
            wq_sb = cpool.tile([128, DCH, 128], F32R)
            nc.sync.dma_start(wq_sb[:], wq_d.rearrange("(a p) m -> p a m", p=128))
            wk_sb = cpool.tile([128, DCH, 128], F32R)
            nc.sync.dma_start(wk_sb[:], wk_d.rearrange("(a p) m -> p a m", p=128))
            wv_sb = cpool.tile([128, DCH, 128], F32R)
            nc.sync.dma_start(wv_sb[:], wv_d.rearrange("(a p) m -> p a m", p=128))
            wo_sb = cpool.tile([128, DCH, 128], F32R)
            cos_sb = cpool.tile([128, S], F32)
            sin_sb = cpool.tile([128, S], F32)

            nc.gpsimd.dma_start(cos_sb[:], cos_d)
            nc.gpsimd.dma_start(sin_sb[:], sin_d)
            nc.gpsimd.dma_start(wo_sb[:], wo_d.rearrange("(a p) m -> p a m", p=128))
            id_sb = cpool.tile([128, 128], F32)
            make_identity(nc, id_sb[:])
            onesf = cpool.tile([128, 1], F32)
            nc.vector.memset(onesf[:], 1.0)

            # ---- persistent activations ----
            qt_all = ppool.tile([128, TOK], F32R)  # RoPE'd Q^T (2 heads stacked)
            kt_all = ppool.tile([128, TOK], F32R)  # RoPE'd K^T
            # token-major V per 128-token block, per-head [64 V | 1 ones] slots
            v_all = ppool.tile([128, 2 * NKB_B, 2 * (HD + 1)], F32R)

            def rope(dst, src_ps, s0):
                """dst[128,ST] (f32r) = src*cos + rotate_half(src)*sin_signed."""
                stg = rpool.tile([128, ST], F32, tag="stg")
                nc.vector.tensor_copy(stg[:], src_ps[:])
                rot = rpool.tile([128, ST], F32, tag="rot")
                for h0 in (0, 64):
                    nc.gpsimd.tensor_copy(
                        rot[h0 : h0 + 32, :], stg[h0 + 32 : h0 + 64, :]
                    )
                    nc.gpsimd.tensor_copy(
                        rot[h0 + 32 : h0 + 64, :], stg[h0 : h0 + 32, :]
                    )
                t1 = rpool.tile([128, ST], F32, tag="t1")
                nc.vector.tensor_mul(t1[:], stg[:], cos_sb[:, s0 : s0 + ST])
                nc.vector.tensor_mul(rot[:], rot[:], sin_sb[:, s0 : s0 + ST])
                nc.vector.tensor_add(dst, t1[:], rot[:])

            # ---- phases 1+2 interleaved: QKV tile then attention per (b, qt) ----
            ag_in = {}
            ag_out = {}
            for b in range(B):
                for qt in range(NQT):
                    ag_in[b, qt] = dpool.tile(
                        [128, ST], F32R, name=f"ag_in{b}_{qt}"
                    )
                    ag_out[b, qt] = dpool.tile(
                        [D, ST], F32R, addr_space="Shared", name=f"ag_out{b}_{qt}"
                    )

            def emit_qkv_tile(st):
                s0 = (st % NQT) * ST  # within-batch position (cos/sin index)
                tok0 = st * ST
                xt_sb = xpool.tile([128, DCH, ST], F32R, tag="xt", name=f"xt{st}")
                xr = xT_d.rearrange("(a p) m -> p a m", p=128)
                if st == 0:  # split so the first matmuls start sooner
                    nc.sync.dma_start(
                        xt_sb[:, 0:4, :], xr[:, 0:4, tok0 : tok0 + ST]
                    )
                    nc.sync.dma_start(
                        xt_sb[:, 4:DCH, :], xr[:, 4:DCH, tok0 : tok0 + ST]
                    )
                else:
                    nc.sync.dma_start(xt_sb[:], xr[:, :, tok0 : tok0 + ST])
                for part, w_sb in (("q", wq_sb), ("k", wk_sb), ("v", wv_sb)):
                    acc = pqkv.tile([128, ST], F32, tag="qkv", name=f"ps_{part}{st}")
                    for dk in range(DCH):
                        nc.tensor.matmul(
                            acc[:],
                            w_sb[:, dk, :],
                            xt_sb[:, dk, :],
                            start=(dk == 0),
                            stop=(dk == DCH - 1),
                        )
                    if part == "q":
                        rope(qt_all[:, tok0 : tok0 + ST], acc, s0)
                    elif part == "k":
                        rope(kt_all[:, tok0 : tok0 + ST], acc, s0)
                    else:
                        vs = rpool.tile([128, ST], F32, tag="stg", name=f"vs{st}")
                        nc.vector.tensor_copy(vs[:], acc[:])
                        slot = st * (ST // KB)
                        tr = pscr.tile([128, ST], F32, tag="scr", name=f"tr{st}")
                        for jj in range(ST // KB):
                            nc.tensor.transpose(
                                tr[:, jj * 128 : (jj + 1) * 128],
                                vs[:, jj * 128 : (jj + 1) * 128],
                                id_sb[:],
                            )
                        trv = tr[:].rearrange("p (j t h) -> p j t h", j=4, t=2)
                        for h in range(HPC):
                            c0 = h * (HD + 1)
                            nc.vector.tensor_copy(
                                v_all[:, slot : slot + 4, c0 : c0 + HD],
                                trv[:, :, h, :],
                            )
                            for jj in range(ST // KB):
                                nc.gpsimd.tensor_copy(
                                    v_all[:, slot + jj, c0 + HD : c0 + HD + 1],
                                    onesf[:],
                                )

            def emit_attention(b, qt):
                q0 = b * S + qt * ST
                nkb = (qt + 1) * (ST // KB)
                o_ps = [
                    po.tile([HD + 1, ST], F32, tag="o", name=f"o{b}_{qt}_{h}")
                    for h in range(HPC)
                ]
                e_tiles = {}

                def emit_scores(kbi):
                    k0 = b * S + kbi * KB
                    for h in range(HPC):
                        stp = pscr.tile(
                            [128, ST], F32, tag="scr", name=f"st{b}_{qt}_{kbi}_{h}"
                        )
                        nc.tensor.matmul(
                            stp[:],
                            kt_all[h * HD : (h + 1) * HD, k0 : k0 + KB],
                            qt_all[h * HD : (h + 1) * HD, q0 : q0 + ST],
                            start=True,
                            stop=True,
                        )
                        e_sb = epool.tile(
                            [128, ST], F32R, tag="e", name=f"e{b}_{qt}_{kbi}_{h}"
                        )
                        nc.scalar.activation(e_sb[:], stp[:], EXP, scale=SCALE)
                        j = kbi - qt * (ST // KB)
                        if j >= 0:
                            # causal: keep where q - k - j*128 >= 0
                            nc.gpsimd.affine_select(
                                out=e_sb[:],
                                in_=e_sb[:],
                                compare_op=mybir.AluOpType.is_ge,
                                fill=0.0,
                                base=-j * KB,
                                pattern=[[1, ST]],
                                channel_multiplier=-1,
                            )
                        e_tiles[kbi, h] = e_sb

                def emit_pv(kbi):
                    slot = b * NKB_B + kbi
                    for h in range(HPC):
                        c0 = h * (HD + 1)
                        nc.tensor.matmul(
                            o_ps[h][:],
                            v_all[:, slot, c0 : c0 + HD + 1],
                            e_tiles.pop((kbi, h))[:],
                            start=(kbi == 0),
                            stop=(kbi == nkb - 1),
                        )

                emit_scores(0)
                for kbi in range(1, nkb):
                    emit_scores(kbi)
                    emit_pv(kbi - 1)
                emit_pv(nkb - 1)
                o2_sb = zpool.tile([128, ST], F32, tag="osb")
                rz = zpool.tile([64, ST], F32, tag="rz")
                for h in range(HPC):
                    nc.any.tensor_copy(
                        o2_sb[h * HD : (h + 1) * HD, :], o_ps[h][0:HD, :]
                    )
                    nc.vector.reciprocal(
                        rz[h * 32 : h * 32 + 1, :], o_ps[h][HD : HD + 1, :]
                    )
                zb = dpool.tile([HPC, ST], F32, tag="rzb", bufs=4, name=f"zb{b}_{qt}")
                nc.sync.dma_start(
                    zb[:], rz[:].rearrange("(a p) m -> a p m", p=32)[:, 0, :]
                )
                bc_sb = zpool.tile([128, ST], F32, tag="bcs")
                nc.sync.dma_start(
                    bc_sb[:],
                    zb[:].rearrange("h (o m) -> h o m", o=1).to_broadcast([HPC, HD, ST]),
                )
                at_sb = apool.tile([128, ST], F32R, tag="at")
                nc.vector.tensor_mul(at_sb[:], o2_sb[:], bc_sb[:])
                nc.sync.dma_start(ag_in[b, qt][:], at_sb[:])

            def emit_ag(b, qt):
                if collective:
                    nc.gpsimd.collective_compute(
                        "AllGather",
                        mybir.AluOpType.bypass,
                        replica_groups=[list(range(NCORE))],
                        ins=[ag_in[b, qt].opt()],
                        outs=[ag_out[b, qt].opt()],
                    )
                else:  # timing-only single-core stand-in (replicate to all slices)
                    nc.sync.dma_start(
                        ag_out[b, qt][:],
                        ag_in[b, qt][:]
                        .rearrange("p (o m) -> o p m", o=1)
                        .to_broadcast([NCORE, 128, ST]),
                    )

            def emit_outproj(b, qt, split=False):
                tt = b * NQT + qt
                ag_sb = gpool.tile([128, DCH, ST], F32R, tag="ag", name=f"ag{tt}")
                agr = ag_out[b, qt].rearrange("(a p) m -> p a m", p=128)
                if split:
                    for fk in range(DCH):
                        nc.sync.dma_start(ag_sb[:, fk, :], agr[:, fk, :])
                else:
                    nc.sync.dma_start(ag_sb[:], agr[:])
                yt_ps = pqkv.tile([128, ST], F32, tag="qkv", name=f"yt{tt}")
                for fk in range(DCH):
                    nc.tensor.matmul(
                        yt_ps[:],
                        wo_sb[:, fk, :],
                        ag_sb[:, fk, :],
                        start=(fk == 0),
                        stop=(fk == DCH - 1),
                    )
                yt_sb = ypool.tile([128, ST], F32, tag="yt")
                nc.vector.tensor_copy(yt_sb[:], yt_ps[:])
                nc.sync.dma_start(yt_d[:, tt * ST : (tt + 1) * ST], yt_sb[:])

            emit_qkv_tile(0)
            for st in range(NST):
                b, qt = st // NQT, st % NQT
                if st + 1 < NST:
                    emit_qkv_tile(st + 1)
                emit_attention(b, qt)
                emit_ag(b, qt)
                if st >= 1:
                    emit_outproj((st - 1) // NQT, (st - 1) % NQT)
            emit_outproj(B - 1, NQT - 1, split=True)

    nc.compile()
    return nc


def _host_tables():
    inv_freq = 1.0 / (ROPE_BASE ** (np.arange(0, HD, 2, dtype=np.float32) / HD))
    t = np.arange(S, dtype=np.float32)
    freqs = np.outer(t, inv_freq)  # [S, 32]
    emb = np.concatenate([freqs, freqs], axis=-1)  # [S, 64]
    cos = np.cos(emb).astype(np.float32)
    sin = np.sin(emb).astype(np.float32)
    sinS = np.concatenate([-sin[:, : HD // 2], sin[:, HD // 2 :]], axis=1)
    cosT2 = np.ascontiguousarray(np.concatenate([cos.T, cos.T], axis=0))  # [128,S]
    sinT2 = np.ascontiguousarray(np.concatenate([sinS.T, sinS.T], axis=0))
    return cosT2, sinT2


def _get_nc():
    if "nc" not in _CACHE:
        _CACHE["nc"] = _build_program()
        _CACHE["tables"] = _host_tables()
    return _CACHE["nc"]


def _make_in_maps(x, w_in, w_out):
    cosT2, sinT2 = _CACHE["tables"]
    xT = np.ascontiguousarray(x.reshape(TOK, D).T)  # [D, TOK]
    in_maps = []
    for c in range(NCORE):
        r = slice(c * 128, (c + 1) * 128)
        in_maps.append(
            {
                "xT": xT,
                "wq": np.ascontiguousarray(w_in[0 * D :][r.start : r.stop].T),
                "wk": np.ascontiguousarray(w_in[1 * D :][r.start : r.stop].T),
                "wv": np.ascontiguousarray(w_in[2 * D :][r.start : r.stop].T),
                "wo": np.ascontiguousarray(w_out[r, :].T),
                "cosT": cosT2,
                "sinT": sinT2,
            }
        )
    return in_maps


def kernel(x: np.ndarray, w_in: np.ndarray, w_out: np.ndarray) -> np.ndarray:
    x = np.asarray(x, dtype=np.float32)
    w_in = np.asarray(w_in, dtype=np.float32)
    w_out = np.asarray(w_out, dtype=np.float32)

    nc = _get_nc()
    in_maps = _make_in_maps(x, w_in, w_out)
    res = run_bass_kernel_spmd(nc, in_maps, core_ids=list(range(NCORE)))
    yT = np.concatenate([res.results[c]["yt"] for c in range(NCORE)], axis=0)
    return np.ascontiguousarray(yT.T).reshape(B, S, D)



# revision 15
# speedup vs baseline: 1.3093x; 1.3093x over previous
"""Trainium2 Bass kernel for nn_CustomAttn: fused QKV + RoPE + causal SDPA + out-proj.

Sharding: tensor-parallel over heads (16 heads / 8 cores = 2 heads/core).
Each core computes QKV for its 2 heads (d-major layouts), RoPE, causal
flash-style attention (scores kept transposed [k, q]), producing attn^T
feature-major [128, tokens] in bf16. An AllGather assembles the full
attn^T [1024, tokens]; each core then computes its 128-row slice of
y^T = w_out @ attn^T.  Host assembles y from the 8 row-slices.

v2 engine plan (vs v1):
- RoPE rotate-half fused into the sin-multiplies on Vector (cross-partition
  operand offsets), no GpSimd copies.
- V ones-columns memset once instead of per-tile GpSimd writes.
- Softmax denominators via vector.reciprocal_approx_fast (5x cheaper).
- Diagonal score blocks trimmed to their valid column range (less PE + exp).
- e/V/attn-out/AllGather payload/out-proj weights in bf16.
- Queue separation: Sync = xt/zb/bc/ag_in/yt + collective triggers;
  Scalar(hwdge) = exp + ag_out loads; Pool = cos/sin/wo preloads + causal
  masks. Out-proj deferred two tiles so its AllGather is already done.
"""
import sys

if "/opt/trn_rl_repo" not in sys.path:
    sys.path.insert(0, "/opt/trn_rl_repo")

import numpy as np

import concourse.bass as bass
import concourse.tile as tile
from concourse import bacc, mybir
from concourse.bass_utils import run_bass_kernel_spmd
from concourse.masks import make_identity

F32 = mybir.dt.float32
F32R = mybir.dt.float32r
BF16 = mybir.dt.bfloat16
EXP = mybir.ActivationFunctionType.Exp

B, S, D, H, HD = 2, 2048, 1024, 16, 64
NCORE = 8
HPC = H // NCORE  # 2 heads per core
TOK = B * S  # 4096 flattened tokens
ST = 512  # s-tile / q-tile width
NST = TOK // ST  # 8
NQT = S // ST  # 4 q-tiles per batch
KB = 128  # k-block
NKB_B = S // KB  # 16 k-blocks per batch
DCH = D // 128  # 8 contraction chunks
SCALE = 1.0 / np.sqrt(HD)
ROPE_BASE = 10000.0

_CACHE: dict = {}

import os

TRIM = os.environ.get("K_TRIM", "1") == "1"
AGBF = os.environ.get("K_AGBF", "0") == "1"  # bf16 AllGather payload corrupts DRAM
RAFAST = os.environ.get("K_RAFAST", "1") == "1"
FUSED_ROPE = os.environ.get("K_FROPE", "1") == "1"


def _build_program(collective: bool = True):
    nc = bacc.Bacc("TRN2", target_bir_lowering=False, debug=False, num_devices=NCORE)

    # ---- DRAM I/O ----
    xT_d = nc.dram_tensor("xT", [D, TOK], F32R, kind="ExternalInput").ap()
    wq_d = nc.dram_tensor("wq", [D, 128], F32R, kind="ExternalInput").ap()
    wk_d = nc.dram_tensor("wk", [D, 128], F32R, kind="ExternalInput").ap()
    wv_d = nc.dram_tensor("wv", [D, 128], F32R, kind="ExternalInput").ap()
    wo_d = nc.dram_tensor("wo", [D, 128], BF16 if AGBF else F32R, kind="ExternalInput").ap()
    cos_d = nc.dram_tensor("cosT", [128, S], F32, kind="ExternalInput").ap()
    sin_d = nc.dram_tensor("sinT", [128, S], F32, kind="ExternalInput").ap()
    yt_d = nc.dram_tensor("yt", [128, TOK], F32, kind="ExternalOutput").ap()

    with tile.TileContext(nc) as tc:
        with (
            tc.tile_pool(name="const", bufs=1) as cpool,
            tc.tile_pool(name="persist", bufs=1) as ppool,
            tc.tile_pool(name="xt", bufs=2) as xpool,
            tc.tile_pool(name="rope", bufs=2) as rpool,
            tc.tile_pool(name="e", bufs=6) as epool,
            tc.tile_pool(name="at", bufs=2) as apool,
            tc.tile_pool(name="rz", bufs=2) as zpool,
            tc.tile_pool(name="agin", bufs=2) as gpool,
            tc.tile_pool(name="yt", bufs=2) as ypool,
            tc.tile_pool(name="pqkv", bufs=2, space="PSUM") as pqkv,
            tc.tile_pool(name="pscr", bufs=3, space="PSUM") as pscr,
            tc.tile_pool(name="ptr", bufs=1, space="PSUM") as ptr,
            tc.tile_pool(name="po", bufs=2, space="PSUM") as po,
            tc.tile_pool(name="dram", bufs=1, space="DRAM") as dpool,
        ):
            # ---- constants / weights ----
            # pool queue first: rope tables (needed ~10us in), wo late.
            cos_sb = cpool.tile([128, S], F32)
            sin_sb = cpool.tile([128, S], F32)
            nc.gpsimd.dma_start(cos_sb[:], cos_d)
            nc.gpsimd.dma_start(sin_sb[:], sin_d)

            # sync queue: wq first (first matmuls), wk/wv after xt0 kick-off.
            wq_sb = cpool.tile([128, DCH, 128], F32R)
            nc.sync.dma_start(wq_sb[:], wq_d.rearrange("(a p) m -> p a m", p=128))

            xr = xT_d.rearrange("(a p) m -> p a m", p=128)
            xt_tiles: dict = {}

            def emit_xt(st):
                tok0 = st * ST
                xt_sb = xpool.tile([128, DCH, ST], F32R, tag="xt", name=f"xt{st}")
                if st == 0:  # fine split so the first matmuls start sooner
                    for c in range(4):
                        nc.sync.dma_start(
                            xt_sb[:, 2 * c : 2 * c + 2, :],
                            xr[:, 2 * c : 2 * c + 2, tok0 : tok0 + ST],
                        )
                else:
                    nc.sync.dma_start(
                        xt_sb[:, 0:4, :], xr[:, 0:4, tok0 : tok0 + ST]
                    )
                    nc.sync.dma_start(
                        xt_sb[:, 4:DCH, :], xr[:, 4:DCH, tok0 : tok0 + ST]
                    )
                xt_tiles[st] = xt_sb

            emit_xt(0)

            wk_sb = cpool.tile([128, DCH, 128], F32R)
            nc.sync.dma_start(wk_sb[:], wk_d.rearrange("(a p) m -> p a m", p=128))
            wv_sb = cpool.tile([128, DCH, 128], F32R)
            nc.sync.dma_start(wv_sb[:], wv_d.rearrange("(a p) m -> p a m", p=128))
            wo_sb = cpool.tile([128, DCH, 128], BF16 if AGBF else F32R)
            nc.gpsimd.dma_start(wo_sb[:], wo_d.rearrange("(a p) m -> p a m", p=128))

            id_sb = cpool.tile([128, 128], BF16)
            make_identity(nc, id_sb[:])

            # ---- persistent activations ----
            qt_all = ppool.tile([128, TOK], F32R)  # RoPE'd Q^T (2 heads stacked)
            kt_all = ppool.tile([128, TOK], F32R)  # RoPE'd K^T
            # token-major V per 128-token block, per-head [64 V | 1 ones] slots
            v_all = ppool.tile([128, 2 * NKB_B, 2 * (HD + 1)], BF16)
            # ones columns written once; PV matmuls then also produce softmax
            # denominators in the 65th output row.
            nc.vector.memset(v_all[:, :, HD : HD + 1], 1.0)
            nc.vector.memset(v_all[:, :, 2 * HD + 1 : 2 * HD + 2], 1.0)

            def rope(dst, ps, s0):
                """dst[128,ST] (f32r) = ps*cos + rotate_half(ps)*sin_signed.

                rotate_half is absorbed into the sin-multiplies via
                partition-offset operands (no copies)."""
                t1 = rpool.tile([128, ST], F32, tag="t1")
                nc.vector.tensor_mul(t1[:], ps[:], cos_sb[:, s0 : s0 + ST])
                rot = rpool.tile([128, ST], F32, tag="rot")
                if FUSED_ROPE:
                    for h0 in (0, 64):
                        nc.vector.tensor_mul(
                            rot[h0 : h0 + 32, :],
                            ps[h0 + 32 : h0 + 64, :],
                            sin_sb[h0 : h0 + 32, s0 : s0 + ST],
                        )
                        nc.vector.tensor_mul(
                            rot[h0 + 32 : h0 + 64, :],
                            ps[h0 : h0 + 32, :],
                            sin_sb[h0 + 32 : h0 + 64, s0 : s0 + ST],
                        )
                else:
                    stg = rpool.tile([128, ST], F32, tag="stg")
                    nc.vector.tensor_copy(stg[:], ps[:])
                    for h0 in (0, 64):
                        nc.gpsimd.tensor_copy(
                            rot[h0 : h0 + 32, :], stg[h0 + 32 : h0 + 64, :]
                        )
                        nc.gpsimd.tensor_copy(
                            rot[h0 + 32 : h0 + 64, :], stg[h0 : h0 + 32, :]
                        )
                    nc.vector.tensor_mul(
                        rot[:], rot[:], sin_sb[:, s0 : s0 + ST]
                    )
                nc.vector.tensor_add(dst, t1[:], rot[:])

            ag_in = {}
            ag_out = {}
            for b in range(B):
                for qt in range(NQT):
                    ag_in[b, qt] = dpool.tile(
                        [128, ST], BF16 if AGBF else F32R, name=f"ag_in{b}_{qt}"
                    )
                    ag_out[b, qt] = dpool.tile(
                        [D, ST],
                        BF16 if AGBF else F32R,
                        addr_space="Shared",
                        name=f"ag_out{b}_{qt}",
                    )

            def emit_qkv(st):
                s0 = (st % NQT) * ST  # within-batch position (cos/sin index)
                tok0 = st * ST
                xt_sb = xt_tiles[st]
                for part, w_sb in (("q", wq_sb), ("k", wk_sb), ("v", wv_sb)):
                    acc = pqkv.tile([128, ST], F32, tag="qkv", name=f"ps_{part}{st}")
                    for dk in range(DCH):
                        nc.tensor.matmul(
                            acc[:],
                            w_sb[:, dk, :],
                            xt_sb[:, dk, :],
                            start=(dk == 0),
                            stop=(dk == DCH - 1),
                        )
                    if part == "q":
                        rope(qt_all[:, tok0 : tok0 + ST], acc, s0)
                    elif part == "k":
                        rope(kt_all[:, tok0 : tok0 + ST], acc, s0)
                    else:
                        vs = rpool.tile([128, ST], BF16, tag="vs", name=f"vs{st}")
                        nc.vector.tensor_copy(vs[:], acc[:])
                        slot = st * (ST // KB)
                        tr = ptr.tile([128, ST], BF16, tag="trv", name=f"tr{st}")
                        for jj in range(ST // KB):
                            nc.tensor.transpose(
                                tr[:, jj * 128 : (jj + 1) * 128],
                                vs[:, jj * 128 : (jj + 1) * 128],
                                id_sb[:],
                            )
                        trv = tr[:].rearrange("p (j t h) -> p j t h", j=4, t=2)
                        for h in range(HPC):
                            c0 = h * (HD + 1)
                            nc.vector.tensor_copy(
                                v_all[:, slot : slot + 4, c0 : c0 + HD],
                                trv[:, :, h, :],
                            )

            def emit_attention(b, qt):
                q0 = b * S + qt * ST
                nkb = (qt + 1) * (ST // KB)
                o_ps = [
                    po.tile([HD + 1, ST], F32, tag="o", name=f"o{b}_{qt}_{h}")
                    for h in range(HPC)
                ]
                e_tiles = {}
                col0 = {}  # valid-column start per k-block (causal trim)

                def emit_scores(kbi):
                    k0 = b * S + kbi * KB
                    j = kbi - qt * (ST // KB)
                    c0 = (max(0, j) * KB if j >= 0 else 0) if TRIM else 0
                    col0[kbi] = c0
                    for h in range(HPC):
                        stp = pscr.tile(
                            [128, ST], F32, tag="scr", name=f"st{b}_{qt}_{kbi}_{h}"
                        )
                        nc.tensor.matmul(
                            stp[:, c0:ST],
                            kt_all[h * HD : (h + 1) * HD, k0 : k0 + KB],
                            qt_all[h * HD : (h + 1) * HD, q0 + c0 : q0 + ST],
                            start=True,
                            stop=True,
                        )
                        e_sb = epool.tile(
                            [128, ST], BF16, tag="e", name=f"e{b}_{qt}_{kbi}_{h}"
                        )
                        nc.scalar.activation(
                            e_sb[:, c0:ST], stp[:, c0:ST], EXP, scale=SCALE
                        )
                        if j >= 0:
                            # causal: within the first 128 valid columns,
                            # keep where (col - c0) - k_local >= 0
                            if TRIM:
                                nc.gpsimd.affine_select(
                                    out=e_sb[:, c0 : c0 + KB],
                                    in_=e_sb[:, c0 : c0 + KB],
                                    compare_op=mybir.AluOpType.is_ge,
                                    fill=0.0,
                                    base=0,
                                    pattern=[[1, KB]],
                                    channel_multiplier=-1,
                                )
                            else:
                                nc.gpsimd.affine_select(
                                    out=e_sb[:],
                                    in_=e_sb[:],
                                    compare_op=mybir.AluOpType.is_ge,
                                    fill=0.0,
                                    base=-j * KB,
                                    pattern=[[1, ST]],
                                    channel_multiplier=-1,
                                )
                        e_tiles[kbi, h] = e_sb

                def emit_pv(kbi):
                    slot = b * NKB_B + kbi
                    c0 = col0[kbi]
                    for h in range(HPC):
                        hc = h * (HD + 1)
                        nc.tensor.matmul(
                            o_ps[h][:, c0:ST],
                            v_all[:, slot, hc : hc + HD + 1],
                            e_tiles.pop((kbi, h))[:, c0:ST],
                            start=(kbi == 0),
                            stop=(kbi == nkb - 1),
                        )

                emit_scores(0)
                for kbi in range(1, nkb):
                    emit_scores(kbi)
                    emit_pv(kbi - 1)
                emit_pv(nkb - 1)

                # softmax denominators -> 1/z (fast approx), broadcast, scale
                zb = dpool.tile([HPC, ST], F32, tag="rzb", bufs=4, name=f"zb{b}_{qt}")
                if RAFAST:
                    # custom DVE op requires partition-0 base: stage z first
                    for h in range(HPC):
                        zs = zpool.tile([1, ST], F32, tag=f"zs{h}")
                        nc.vector.tensor_copy(zs[:], o_ps[h][HD : HD + 1, :])
                        rzt = zpool.tile([1, ST], F32, tag=f"rza{h}")
                        nc.vector.reciprocal_approx_fast(rzt[:], zs[:])
                        nc.sync.dma_start(zb[h : h + 1, :], rzt[:])
                else:
                    rz = zpool.tile([64, ST], F32, tag="rz")
                    for h in range(HPC):
                        nc.vector.reciprocal(
                            rz[h * 32 : h * 32 + 1, :], o_ps[h][HD : HD + 1, :]
                        )
                    nc.sync.dma_start(
                        zb[:], rz[:].rearrange("(a p) m -> a p m", p=32)[:, 0, :]
                    )
                bc_sb = zpool.tile([128, ST], F32, tag="bcs")
                nc.sync.dma_start(
                    bc_sb[:],
                    zb[:].rearrange("h (o m) -> h o m", o=1).to_broadcast([HPC, HD, ST]),
                )
                at_sb = apool.tile([128, ST], BF16 if AGBF else F32R, tag="at")
                for h in range(HPC):
                    nc.vector.tensor_mul(
                        at_sb[h * HD : (h + 1) * HD, :],
                        o_ps[h][0:HD, :],
                        bc_sb[h * HD : (h + 1) * HD, :],
                    )
                nc.sync.dma_start(ag_in[b, qt][:], at_sb[:])
                if collective:
                    nc.gpsimd.collective_compute(
                        "AllGather",
                        mybir.AluOpType.bypass,
                        replica_groups=[list(range(NCORE))],
                        ins=[ag_in[b, qt].opt()],
                        outs=[ag_out[b, qt].opt()],
                    )
                else:  # timing-only single-core stand-in
                    nc.sync.dma_start(
                        ag_out[b, qt][:],
                        ag_in[b, qt][:]
                        .rearrange("p (o m) -> o p m", o=1)
                        .to_broadcast([NCORE, 128, ST]),
                    )

            def emit_outproj(b, qt, split=False):
                tt = b * NQT + qt
                ag_sb = gpool.tile(
                    [128, DCH, ST], BF16 if AGBF else F32R, tag="ag", name=f"ag{tt}"
                )
                agr = ag_out[b, qt].rearrange("(a p) m -> p a m", p=128)
                if split:
                    for fk in range(DCH):
                        nc.scalar.dma_start(ag_sb[:, fk, :], agr[:, fk, :])
                else:
                    nc.scalar.dma_start(ag_sb[:], agr[:])
                yt_ps = pqkv.tile([128, ST], F32, tag="qkv", name=f"yt{tt}")
                for fk in range(DCH):
                    nc.tensor.matmul(
                        yt_ps[:],
                        wo_sb[:, fk, :],
                        ag_sb[:, fk, :],
                        start=(fk == 0),
                        stop=(fk == DCH - 1),
                    )
                yt_sb = ypool.tile([128, ST], F32, tag="yt")
                nc.vector.tensor_copy(yt_sb[:], yt_ps[:])
                nc.sync.dma_start(yt_d[:, tt * ST : (tt + 1) * ST], yt_sb[:])

            emit_qkv(0)
            for st in range(NST):
                b, qt = st // NQT, st % NQT
                if st + 1 < NST:
                    emit_xt(st + 1)
                    emit_qkv(st + 1)
                emit_attention(b, qt)
                if st >= 2:
                    emit_outproj((st - 2) // NQT, (st - 2) % NQT)
            emit_outproj((NST - 2) // NQT, (NST - 2) % NQT)
            emit_outproj(B - 1, NQT - 1, split=True)

    nc.compile()
    return nc


def _host_tables():
    import ml_dtypes

    inv_freq = 1.0 / (ROPE_BASE ** (np.arange(0, HD, 2, dtype=np.float32) / HD))
    t = np.arange(S, dtype=np.float32)
    freqs = np.outer(t, inv_freq)  # [S, 32]
    emb = np.concatenate([freqs, freqs], axis=-1)  # [S, 64]
    cos = np.cos(emb).astype(np.float32)
    sin = np.sin(emb).astype(np.float32)
    sinS = np.concatenate([-sin[:, : HD // 2], sin[:, HD // 2 :]], axis=1)
    cosT2 = np.ascontiguousarray(np.concatenate([cos.T, cos.T], axis=0))  # [128,S]
    sinT2 = np.ascontiguousarray(np.concatenate([sinS.T, sinS.T], axis=0))
    return cosT2, sinT2


def _get_nc():
    if "nc" not in _CACHE:
        _CACHE["nc"] = _build_program()
        _CACHE["tables"] = _host_tables()
    return _CACHE["nc"]


def _make_in_maps(x, w_in, w_out):
    import ml_dtypes

    cosT2, sinT2 = _CACHE["tables"]
    xT = np.ascontiguousarray(x.reshape(TOK, D).T)  # [D, TOK]
    in_maps = []
    for c in range(NCORE):
        r = slice(c * 128, (c + 1) * 128)
        in_maps.append(
            {
                "xT": xT,
                "wq": np.ascontiguousarray(w_in[0 * D :][r.start : r.stop].T),
                "wk": np.ascontiguousarray(w_in[1 * D :][r.start : r.stop].T),
                "wv": np.ascontiguousarray(w_in[2 * D :][r.start : r.stop].T),
                "wo": np.ascontiguousarray(
                    w_out[r, :].T.astype(ml_dtypes.bfloat16)
                    if AGBF
                    else w_out[r, :].T
                ),
                "cosT": cosT2,
                "sinT": sinT2,
            }
        )
    return in_maps


def kernel(x: np.ndarray, w_in: np.ndarray, w_out: np.ndarray) -> np.ndarray:
    x = np.asarray(x, dtype=np.float32)
    w_in = np.asarray(w_in, dtype=np.float32)
    w_out = np.asarray(w_out, dtype=np.float32)

    nc = _get_nc()
    in_maps = _make_in_maps(x, w_in, w_out)
    res = run_bass_kernel_spmd(nc, in_maps, core_ids=list(range(NCORE)))
    yT = np.concatenate([res.results[c]["yt"] for c in range(NCORE)], axis=0)
    return np.ascontiguousarray(yT.T).reshape(B, S, D)
